# revision 9
# baseline (speedup 1.0000x reference)
"""Detection postprocess (decode + top-60 + per-image NMS) on 8 TRN2 NeuronCores.

Data-parallel over the batch: 256 images -> 32 per core. Per core, one raw-Bass
program:

  DVE   : per-chunk top-8 via max/max_index (value + exact position in one
          instruction pair), per-image top-64 over the 1024-slot pool via 8
          rounds of max/max_index/match_replace, then an op-minimized 20-step
          NMS on logits (one image per partition, 32 in lockstep): per step the
          selected candidate's lo/hi/volume channels are fetched with a single
          one-hot multiply+reduce over a 7-channel table; output rows are
          assembled in one batched pass after the loop.
  GPSIMD: all DMAs + two indirect_copy gather rounds (pool gidx at the top-64
          pool positions, then off/sh/anchor channel rows at the 64 global
          indices, 8 images per 16-partition-group call).
  ACT   : sigmoid of the top-64 logits.

Anchors (pre-scaled by the stride) and the small index-arithmetic constants are
inline_tensor'd into the NEFF - no runtime transfer. The compiled executable is
cached at module level, so repeat kernel() calls skip tracing/compile/load.

Ordering is exact f32 logit order; bitwise-equal logit ties may resolve
differently from jax top_k (duplicated instead of distinct rows) - vanishingly
rare and within the correctness gate.

NOTE: the indirect_copy data tiles (DG0/DG1/GD1) must sit at high SBUF
addresses; with the gather table at a low address, large gather indices crash
the device (empirically: the Q7 gather ucode mishandles them). Allocation
order below keeps them above the proven address range.
"""

import numpy as np

import concourse.bass as bass
from concourse import mybir

dt = mybir.dt
Alu = mybir.AluOpType
AF = mybir.ActivationFunctionType
Ax = mybir.AxisListType

B = 32            # images per core
N = 13824         # anchors per image (24^3)
CH = 108          # chunk length
Q = 128           # chunks per image
TOP = 64
NMSK = 20
NEG = -1e9
NEGINF = -1e30
L0 = float(np.float32(np.log(np.float32(0.15) / np.float32(0.85))))  # logit threshold
THP = float(np.float32(0.05) / np.float32(1.05))  # iou>th <=> inter > THP*(v1+v2)


def build_nc(sim_friendly=False):
    nc = bass.Bass("TRN2", target_bir_lowering=False, debug=False, num_devices=8)

    cls = nc.declare_dram_parameter("cls", [B, N], dt.float32, isOutput=False)
    off = nc.declare_dram_parameter("off", [B, 3, N], dt.float32, isOutput=False)
    sh = nc.declare_dram_parameter("sh", [B, 3, N], dt.float32, isOutput=False)
    outp = nc.declare_dram_parameter("out", [B, 60, 8], dt.float32, isOutput=True)

    # --- inline consts (baked into the NEFF, no runtime transfer) ---
    n = np.arange(N)
    anc4 = np.stack([n // 576, (n // 24) % 24, n % 24]).astype(np.float32) * 4.0
    c_anc4 = nc.inline_tensor(
        np.broadcast_to(anc4[:, None, :], (3, 8, N)).copy(), "c_anc4")  # [3, 8, N]
    c_chb = nc.inline_tensor(
        (np.arange(128, dtype=np.float32) * CH).reshape(128, 1), "c_chb")
    c_iota = nc.inline_tensor(
        np.broadcast_to(np.arange(TOP, dtype=np.float32), (B, TOP)).copy(), "c_iota")

    # --- DRAM scratch ---
    poold = nc.dram_tensor("poold", [B, Q * 8], dt.float32)
    gipd = nc.dram_tensor("gipd", [B, Q * 8], dt.float32)
    scr_pw = nc.dram_tensor("scr_pw", [B, TOP], dt.uint16)
    scr_gw = nc.dram_tensor("scr_gw", [B, TOP], dt.uint16)
    scr_o1 = nc.dram_tensor("scr_o1", [128, 4 * TOP], dt.float32)
    scr_g2 = nc.dram_tensor("scr_g2", [4, 128, TOP], dt.float32)

    # --- SBUF ---
    T1 = nc.alloc_sbuf_tensor("T1", [128, B * CH], dt.float32)     # [q, (b j)]
    CHB = nc.alloc_sbuf_tensor("CHB", [128, 1], dt.float32)
    V1 = nc.alloc_sbuf_tensor("V1", [128, B * 8], dt.float32)
    IW = nc.alloc_sbuf_tensor("IW", [128, B * 8], dt.uint16)
    GIDXF = nc.alloc_sbuf_tensor("GIDXF", [128, B * 8], dt.float32)
    POOL = nc.alloc_sbuf_tensor("POOL", [B, Q * 8], dt.float32)
    VTOP = nc.alloc_sbuf_tensor("VTOP", [B, TOP], dt.float32)
    PIDX = nc.alloc_sbuf_tensor("PIDX", [B, TOP], dt.uint16)
    PIDXW = nc.alloc_sbuf_tensor("PIDXW", [B, TOP], dt.uint16)    # wrapped pidx
    G64F = nc.alloc_sbuf_tensor("G64F", [B, TOP], dt.float32)     # gidx (f32)
    GIDXW = nc.alloc_sbuf_tensor("GIDXW", [B, TOP], dt.uint16)    # wrapped gidx
    # gather data tiles: keep at high SBUF addresses (see module docstring)
    TMPO = nc.alloc_sbuf_tensor("TMPO", [B, NMSK * 7 * TOP], dt.float32)
    OHA = nc.alloc_sbuf_tensor("OHA", [B, NMSK * TOP], dt.float32)
    GSO = nc.alloc_sbuf_tensor("GSO", [B, 7 * TOP], dt.float32)   # SIG|C3|S3
    GS7 = nc.alloc_sbuf_tensor("GS7", [B, 7 * TOP], dt.float32)   # LO3|HI3|V2S
    _pad0 = nc.alloc_sbuf_tensor("_pad0", [128, 96], dt.float32)
    DG0 = nc.alloc_sbuf_tensor("DG0", [128, N], dt.float32)
    DG1 = nc.alloc_sbuf_tensor("DG1", [128, N], dt.float32)
    GD1 = nc.alloc_sbuf_tensor("GD1", [128, Q * 8], dt.float32)
    PW1 = nc.alloc_sbuf_tensor("PW1", [128, 32], dt.uint16)
    O1 = nc.alloc_sbuf_tensor("O1", [128, 4 * TOP], dt.float32)
    PW2 = nc.alloc_sbuf_tensor("PW2", [128, 32], dt.uint16)
    G2 = nc.alloc_sbuf_tensor("G2", [128, TOP], dt.float32)
    RAW = nc.alloc_sbuf_tensor("RAW", [B, 9 * TOP], dt.float32)   # off3|sh3|anc3
    LOTI = nc.alloc_sbuf_tensor("LOTI", [B, 3 * TOP], dt.float32)  # interleaved
    HITI = nc.alloc_sbuf_tensor("HITI", [B, 3 * TOP], dt.float32)
    TMPV = nc.alloc_sbuf_tensor("TMPV", [B, TOP], dt.float32)
    W = nc.alloc_sbuf_tensor("W", [B, TOP], dt.float32)
    NEGT = nc.alloc_sbuf_tensor("NEGT", [B, TOP], dt.float32)
    MU8 = nc.alloc_sbuf_tensor("MU8", [B, TOP], dt.uint8)
    IOTA = nc.alloc_sbuf_tensor("IOTA", [B, TOP], dt.float32)
    NIDX = nc.alloc_sbuf_tensor("NIDX", [B, 8], dt.uint16)
    NIDXF = nc.alloc_sbuf_tensor("NIDXF", [B, 1], dt.float32)
    M8A = nc.alloc_sbuf_tensor("M8A", [B, NMSK * 8], dt.float32)
    TMP7 = nc.alloc_sbuf_tensor("TMP7", [B, 7 * TOP], dt.float32)
    G7 = nc.alloc_sbuf_tensor("G7", [B, 8], dt.float32)
    T1M = nc.alloc_sbuf_tensor("T1M", [B, 3 * TOP], dt.float32)
    T2M = nc.alloc_sbuf_tensor("T2M", [B, 3 * TOP], dt.float32)
    DIF = nc.alloc_sbuf_tensor("DIF", [B, 3 * TOP], dt.float32)
    DIF0 = nc.alloc_sbuf_tensor("DIF0", [B, 3 * TOP], dt.float32)
    INT2 = nc.alloc_sbuf_tensor("INT2", [B, TOP], dt.float32)
    INTER = nc.alloc_sbuf_tensor("INTER", [B, TOP], dt.float32)
    RR = nc.alloc_sbuf_tensor("RR", [B, TOP], dt.float32)
    SUPM = nc.alloc_sbuf_tensor("SUPM", [B, TOP], dt.uint8)
    G20 = nc.alloc_sbuf_tensor("G20", [B, NMSK * 7], dt.float32)
    VV20 = nc.alloc_sbuf_tensor("VV20", [B, NMSK], dt.float32)
    TQ = nc.alloc_sbuf_tensor("TQ", [B, NMSK * 7], dt.float32)
    OUTT = nc.alloc_sbuf_tensor("OUTT", [B, 60 * 8], dt.float32)
    DMY = nc.alloc_sbuf_tensor("DMY", [B, 4], dt.float32)

    semD = nc.alloc_semaphore("semD")   # gpsimd DMA completions (16 each)
    semB0 = nc.alloc_semaphore("semB0")  # bulk DG0 loads (sync engine)
    semB1 = nc.alloc_semaphore("semB1")  # bulk DG1 loads (scalar engine)
    semG = nc.alloc_semaphore("semG")   # R2 gather milestones (buffer free)
    semV = nc.alloc_semaphore("semV")   # DVE milestones
    semA = nc.alloc_semaphore("semA")   # ACT milestone
    semM = nc.alloc_semaphore("semM")   # sim-only: memsets before bulk loads

    ctr = {"d": 0, "b0": 0, "b1": 0}
    marks = {}

    def dma(g, out_ap, in_ap):
        g.dma_start(out=out_ap, in_=in_ap).then_inc(semD, 16)
        ctr["d"] += 16

    def dmab(e, key, sem, out_ap, in_ap):
        e.dma_start(out=out_ap, in_=in_ap).then_inc(sem, 16)
        ctr[key] += 16

    with nc.Block() as block:

        def dg_load_boxes(e, key, sem, call):
            DG = DG0 if call % 2 == 0 else DG1
            if sim_friendly:
                for i in range(8):
                    dmab(e, key, sem, DG[16 * i : 16 * i + 3, :], off[8 * call + i, :, :])
                    dmab(e, key, sem, DG[16 * i + 3 : 16 * i + 6, :], sh[8 * call + i, :, :])
            else:
                for c in range(3):
                    dmab(e, key, sem, DG[c : 128 : 16, :], off[8 * call : 8 * call + 8, c, :])
                    dmab(e, key, sem, DG[3 + c : 128 : 16, :], sh[8 * call : 8 * call + 8, c, :])

        def dg_load_anchors(e, key, sem, DG):
            if sim_friendly:
                for i in range(8):
                    dmab(e, key, sem, DG[16 * i + 6 : 16 * i + 9, :], c_anc4[:, i, :])
            else:
                for c in range(3):
                    dmab(e, key, sem, DG[6 + c : 128 : 16, :], c_anc4[c, :, :])

        def wrapped(rows):
            return rows.rearrange("m (r j) -> m r j", r=16)

        @block.gpsimd
        def _(g):
            if sim_friendly:
                # the sim flags reads of never-written partitions inside the
                # indirect_copy data tiles; zero them before the loader
                # engines start (semM gates them). HW skips this.
                g.memset(DG0[:], 0.0)
                g.memset(DG1[:], 0.0)
                g.memset(GD1[:], 0.0).then_inc(semM, 1)
            # consts + logits -> chunk layout
            dma(g, CHB[:], c_chb[:])
            dma(g, IOTA[:], c_iota[:])
            dma(g, T1[:], cls[:].rearrange("b (q j) -> q b j", q=Q))
            marks["d_in"] = ctr["d"]

            # stage-A results -> pool layouts (DRAM bounce)
            g.wait_ge(semV, 1)
            dma(g, poold[:].rearrange("b (q k) -> q b k", q=Q),
                V1[:].rearrange("q (b k) -> q b k", b=B))
            dma(g, gipd[:].rearrange("b (q k) -> q b k", q=Q),
                GIDXF[:].rearrange("q (b k) -> q b k", b=B))
            g.wait_ge(semD, ctr["d"])
            dma(g, POOL[:], poold[:])
            marks["d_pool"] = ctr["d"]

            # R1: gather gidx at the top-64 pool positions (8 images/call)
            g.wait_ge(semV, 2)
            dma(g, scr_pw[:], PIDXW[:])
            g.wait_ge(semD, ctr["d"])
            for c in range(4):
                if sim_friendly:
                    for i in range(8):
                        dma(g, GD1[16 * i : 16 * i + 1, :],
                            gipd[8 * c + i : 8 * c + i + 1, :])
                else:
                    dma(g, GD1[0:128:16, :], gipd[8 * c : 8 * c + 8, :])
                dma(g, PW1[:, 0:4], wrapped(scr_pw[8 * c : 8 * c + 8, :]))
                g.wait_ge(semD, ctr["d"])
                g.indirect_copy(O1[:, c * TOP : (c + 1) * TOP], GD1[:], PW1[:, 0:4], True)
            dma(g, scr_o1[:], O1[:])
            g.wait_ge(semD, ctr["d"])
            dma(g, G64F[:], scr_o1[:].rearrange("(g w) (c k) -> c g w k", w=16, c=4)[:, :, 0:1, :])
            marks["d_g64"] = ctr["d"]

            # R2: gather off/sh/anc channel rows at the 64 global indices
            g.wait_ge(semV, 3)
            dma(g, scr_gw[:], GIDXW[:])
            g.wait_ge(semD, ctr["d"])
            nb_stage = 24 if sim_friendly else 9   # anchors + one call, per engine
            nb_call = 16 if sim_friendly else 6
            for c in range(4):
                DG = DG0 if c % 2 == 0 else DG1
                semB = semB0 if c % 2 == 0 else semB1
                need = (nb_stage + (nb_call if c >= 2 else 0)) * 16
                dma(g, PW2[:, 0:4], wrapped(scr_gw[8 * c : 8 * c + 8, :]))
                g.wait_ge(semD, ctr["d"])
                g.wait_ge(semB, need)
                g.indirect_copy(G2[:], DG[:], PW2[:, 0:4], True)
                g.sem_inc(semG, 1)
                dma(g, scr_g2[c, :, :], G2[:])
            g.wait_ge(semD, ctr["d"])
            dma(g, RAW[:], scr_g2[:].rearrange("c (g w) k -> c g w k", w=16)[:, :, 0:9, :])
            marks["d_raw"] = ctr["d"]

            # output
            g.wait_ge(semV, 4)
            dma(g, outp[:], OUTT[:])
            g.wait_ge(semD, ctr["d"])

        @block.vector
        def _(v):
            # ---- stage A: per-chunk top-8 values + positions ----
            v.wait_ge(semD, marks["d_in"])
            for b in range(B):
                v.max(V1[:, b * 8 : (b + 1) * 8], T1[:, b * CH : (b + 1) * CH])
            v.drain()
            for b in range(B):
                v.max_index(IW[:, b * 8 : (b + 1) * 8], V1[:, b * 8 : (b + 1) * 8],
                            T1[:, b * CH : (b + 1) * CH])
            v.drain()
            v.tensor_copy(GIDXF[:], IW[:])                                  # u16->f32
            v.drain()
            v.tensor_scalar(GIDXF[:], GIDXF[:], CHB[:, 0:1], None, Alu.add)
            v.drain()
            v.memset(NEGT[:], NEG).then_inc(semV, 1)

            # ---- stage C: pool top-64 (values + pool positions) ----
            v.wait_ge(semD, marks["d_pool"])
            for r in range(8):
                v.max(VTOP[:, r * 8 : (r + 1) * 8], POOL[:])
                v.drain()
                v.max_index(PIDX[:, r * 8 : (r + 1) * 8],
                            VTOP[:, r * 8 : (r + 1) * 8], POOL[:])
                v.drain()
                v.match_replace(POOL[:], VTOP[:, r * 8 : (r + 1) * 8], POOL[:],
                                NEGINF)
                v.drain()
            # wrapped pool positions for the R1 indirect_copy
            v.tensor_copy(PIDXW[:].rearrange("m (r j) -> m r j", j=4),
                          PIDX[:].rearrange("m (j r) -> m r j", r=16))
            # candidate list W (logits, masked)
            v.tensor_copy(W[:], VTOP[:])
            v.tensor_scalar(MU8[:], VTOP[:], L0, None, Alu.is_le)
            v.drain()
            v.copy_predicated(W[:], MU8[:], NEGT[:])
            v.memset(W[:, 60:TOP], NEG)
            v.drain()
            v.memset(DMY[:, 0:1], 0.0).then_inc(semV, 1)

            # ---- wrapped gidx for the R2 indirect_copy ----
            v.wait_ge(semD, marks["d_g64"])
            v.tensor_copy(GIDXW[:].rearrange("m (r j) -> m r j", j=4),
                          G64F[:].rearrange("m (j r) -> m r j", r=16))
            v.drain()
            v.memset(DMY[:, 1:2], 0.0).then_inc(semV, 1)

            # ---- decode gathered channels ----
            v.wait_ge(semD, marks["d_raw"])
            # centers C3 = off*4 + anc4  (GSO = SIG|C3|S3); S3 copied from RAW
            v.scalar_tensor_tensor(GSO[:, TOP : 4 * TOP], RAW[:, 0 : 3 * TOP], 4.0,
                                   RAW[:, 6 * TOP : 9 * TOP], Alu.mult, Alu.add)
            v.tensor_copy(GSO[:, 4 * TOP : 7 * TOP], RAW[:, 3 * TOP : 6 * TOP])
            v.drain()
            # v2s = -THP * s0*s1*s2  (GS7 = LO3|HI3|V2S)
            v.tensor_tensor(TMPV[:], GSO[:, 4 * TOP : 5 * TOP],
                            GSO[:, 5 * TOP : 6 * TOP], Alu.mult)
            v.drain()
            v.scalar_tensor_tensor(GS7[:, 6 * TOP : 7 * TOP], TMPV[:], -THP,
                                   GSO[:, 6 * TOP : 7 * TOP], Alu.mult, Alu.mult)
            # lo/hi (channel-major)
            v.scalar_tensor_tensor(GS7[:, 0 : 3 * TOP], GSO[:, 4 * TOP : 7 * TOP],
                                   -0.5, GSO[:, TOP : 4 * TOP], Alu.mult, Alu.add)
            v.scalar_tensor_tensor(GS7[:, 3 * TOP : 6 * TOP], GSO[:, 4 * TOP : 7 * TOP],
                                   0.5, GSO[:, TOP : 4 * TOP], Alu.mult, Alu.add)
            v.drain()
            # interleaved copies for the per-candidate min/max
            v.tensor_copy(LOTI[:].rearrange("b (t c) -> b c t", c=3),
                          GS7[:, 0 : 3 * TOP].rearrange("b (c t) -> b c t", c=3))
            v.tensor_copy(HITI[:].rearrange("b (t c) -> b c t", c=3),
                          GS7[:, 3 * TOP : 6 * TOP].rearrange("b (c t) -> b c t", c=3))
            v.wait_ge(semA, 1)   # GSO sigmoid channel (ACT)
            v.drain()

            loti3 = LOTI[:].rearrange("b (t c) -> b t c", c=3)
            hiti3 = HITI[:].rearrange("b (t c) -> b t c", c=3)

            # ---- NMS: 20 steps ----
            for s in range(NMSK):
                v.max(M8A[:, s * 8 : (s + 1) * 8], W[:])
                v.drain()
                v.max_index(NIDX[:], M8A[:, s * 8 : (s + 1) * 8], W[:])
                v.drain()
                v.tensor_copy(NIDXF[:], NIDX[:, 0:1])
                v.drain()
                oh = OHA[:, s * TOP : (s + 1) * TOP]
                v.tensor_scalar(oh, IOTA[:], NIDXF[:, 0:1], None, Alu.is_equal)
                v.drain()
                ohb7 = oh.rearrange("b (o t) -> b o t", o=1).broadcast_to((B, 7, TOP))
                v.tensor_tensor(TMP7[:].rearrange("b (c t) -> b c t", c=7),
                                GS7[:].rearrange("b (c t) -> b c t", c=7),
                                ohb7, Alu.mult)
                v.drain()
                v.tensor_reduce(G7[:, 0:7], TMP7[:].rearrange("b (c t) -> b c t", c=7),
                                Ax.X, Alu.add)
                v.drain()
                blob = G7[:, 0:3].rearrange("b (o c) -> b o c", o=1).broadcast_to((B, TOP, 3))
                bhib = G7[:, 3:6].rearrange("b (o c) -> b o c", o=1).broadcast_to((B, TOP, 3))
                v.tensor_tensor(T1M[:].rearrange("b (t c) -> b t c", c=3), hiti3,
                                bhib, Alu.min)
                v.tensor_tensor(T2M[:].rearrange("b (t c) -> b t c", c=3), loti3,
                                blob, Alu.max)
                v.drain()
                v.tensor_tensor(DIF[:], T1M[:], T2M[:], Alu.subtract)
                v.drain()
                v.tensor_scalar(DIF0[:], DIF[:], 0.0, None, Alu.max)
                v.drain()
                dif3 = DIF0[:].rearrange("b (t c) -> b t c", c=3)
                v.tensor_tensor(INT2[:], dif3[:, :, 0], dif3[:, :, 1], Alu.mult)
                v.drain()
                v.tensor_tensor(INTER[:], INT2[:], dif3[:, :, 2], Alu.mult)
                v.drain()
                v.scalar_tensor_tensor(RR[:], GS7[:, 6 * TOP : 7 * TOP],
                                       G7[:, 6:7], INTER[:], Alu.add, Alu.add)
                v.drain()
                v.scalar_tensor_tensor(SUPM[:], RR[:], 0.0, oh, Alu.is_gt, Alu.add)
                v.drain()
                v.copy_predicated(W[:], SUPM[:], NEGT[:])
                v.drain()

            # ---- batched output rows ----
            gsob = GSO[:].rearrange("b (o c t) -> b o c t", o=1, c=7).broadcast_to((B, NMSK, 7, TOP))
            ohab = OHA[:].rearrange("b (s o t) -> b s o t", s=NMSK, o=1).broadcast_to((B, NMSK, 7, TOP))
            v.tensor_tensor(TMPO[:].rearrange("b (s c t) -> b s c t", s=NMSK, c=7),
                            gsob, ohab, Alu.mult)
            v.tensor_scalar(VV20[:], M8A[:].rearrange("b (s k) -> b s k", k=8)[:, :, 0:1],
                            -5e8, None, Alu.is_gt)
            v.drain()
            v.tensor_reduce(G20[:], TMPO[:].rearrange("b (sc t) -> b sc t", t=TOP),
                            Ax.X, Alu.add)
            v.drain()
            vvb = VV20[:].rearrange("b (s o) -> b s o", o=1).broadcast_to((B, NMSK, 7))
            v.scalar_tensor_tensor(TQ[:].rearrange("b (s c) -> b s c", c=7),
                                   G20[:].rearrange("b (s c) -> b s c", c=7),
                                   1.0, vvb, Alu.add, Alu.mult)
            v.tensor_scalar(OUTT[:].rearrange("b (s k) -> b s k", k=8)[:, 0:NMSK, 0:1],
                            VV20[:].rearrange("b (s o) -> b s o", o=1),
                            2.0, -1.0, Alu.mult, Alu.add)
            v.memset(OUTT[:, NMSK * 8 : 60 * 8], -1.0)
            v.drain()
            v.tensor_scalar(OUTT[:].rearrange("b (s k) -> b s k", k=8)[:, 0:NMSK, 1:8],
                            TQ[:].rearrange("b (s c) -> b s c", c=7),
                            1.0, None, Alu.subtract)
            v.drain()
            v.memset(DMY[:, 2:3], 0.0).then_inc(semV, 1)

        @block.sync
        def _(sp):
            if sim_friendly:
                sp.wait_ge(semM, 1)
            dg_load_anchors(sp, "b0", semB0, DG0)
            dg_load_boxes(sp, "b0", semB0, 0)
            sp.wait_ge(semG, 1)          # gather 0 done -> DG0 free
            dg_load_boxes(sp, "b0", semB0, 2)

        @block.scalar
        def _(a):
            if sim_friendly:
                a.wait_ge(semM, 1)
            dg_load_anchors(a, "b1", semB1, DG1)
            dg_load_boxes(a, "b1", semB1, 1)
            a.wait_ge(semV, 2)
            a.activation(GSO[:, 0:TOP], VTOP[:], AF.Sigmoid).then_inc(semA, 1)
            a.wait_ge(semG, 2)           # gather 1 done -> DG1 free
            dg_load_boxes(a, "b1", semB1, 3)

    return nc


_CACHE = {}


def _get_nc():
    if "nc" not in _CACHE:
        _CACHE["nc"] = build_nc()
    return _CACHE["nc"]


def _make_runner(nc, n_cores=8):
    """Persistent jitted executable: trace/compile/load once, reuse across calls.

    Takes the FULL (256-row) input arrays directly - they are already the
    concatenation of the 8 per-core shards, so no per-call copies are needed.
    """
    import jax
    from jax.sharding import Mesh, PartitionSpec
    from jax.experimental.shard_map import shard_map
    from concourse import bass2jax

    bass2jax.install_neuronx_cc_hook()
    partition_name = nc.partition_id_tensor.name if nc.partition_id_tensor else None
    in_names, out_names, out_avals = [], [], []
    for alloc in nc.m.functions[0].allocations:
        if not isinstance(alloc, mybir.MemoryLocationSet):
            continue
        name = alloc.memorylocations[0].name
        if alloc.kind == "ExternalInput":
            if name != partition_name:
                in_names.append(name)
        elif alloc.kind == "ExternalOutput":
            out_names.append(name)
            out_avals.append(jax.core.ShapedArray(
                tuple(alloc.tensor_shape), mybir.dt.np(alloc.dtype)))
    n_params = len(in_names)
    n_outs = len(out_avals)
    all_in = in_names + out_names + ([partition_name] if partition_name else [])

    def _body(*args):
        operands = list(args)
        if partition_name is not None:
            operands.append(bass2jax.partition_id_tensor())
        outs = bass2jax._bass_exec_p.bind(
            *operands,
            out_avals=tuple(out_avals),
            in_names=tuple(all_in),
            out_names=tuple(out_names),
            lowering_input_output_aliases=(),
            sim_require_finite=True,
            sim_require_nnan=True,
            nc=nc,
        )
        return tuple(outs)

    devices = jax.devices()[:n_cores]
    mesh = Mesh(np.asarray(devices), ("core",))
    in_specs = (PartitionSpec("core"),) * (n_params + n_outs)
    out_specs = (PartitionSpec("core"),) * n_outs
    donate = tuple(range(n_params, n_params + n_outs))
    sharded = jax.jit(
        shard_map(_body, mesh=mesh, in_specs=in_specs, out_specs=out_specs,
                  check_rep=False),
        donate_argnums=donate,
        keep_unused=True,
    )

    def run(full_inputs):
        """full_inputs: dict name -> full (n_cores*rows, ...) array."""
        concat_in = [full_inputs[name] for name in in_names]
        concat_zeros = [
            np.zeros((n_cores * a.shape[0], *a.shape[1:]), a.dtype) for a in out_avals
        ]
        out_arrs = sharded(*concat_in, *concat_zeros)
        return {name: np.asarray(out_arrs[i]) for i, name in enumerate(out_names)}

    return run


def _get_runner():
    if "run" not in _CACHE:
        _CACHE["run"] = _make_runner(_get_nc())
    return _CACHE["run"]


def kernel(cls_out, shape_out, offset_out):
    cls = np.ascontiguousarray(cls_out.reshape(256, N), dtype=np.float32)
    off = np.ascontiguousarray(offset_out.reshape(256, 3, N), dtype=np.float32)
    sh = np.ascontiguousarray(shape_out.reshape(256, 3, N), dtype=np.float32)
    run = _get_runner()
    out = run({"cls": cls, "off": off, "sh": sh})["out"]
    return out.reshape(256, 60, 8).astype(np.float32)
